# revision 71
# baseline (speedup 1.0000x reference)
"""BotRGCN (4 shared RGCN layers) on 8 TRN2 NeuronCores via Bass/Tile.

Strategy (sharding_hint): nodes sharded across 8 cores (6250 each, padded to
6656 = 13*512); edges partitioned by destination core and sorted by
(dst_local, rel) segment; per layer an AllGather replicates the row-major x
table (fp16) to every core's DRAM, then each core dma_gathers its edges'
source rows and computes segment means via PE matmuls against 0/1*(1/cnt)
membership matrices (built host-side; graph is static so the full tiling
is baked into the compiled program, identical across cores). Per-relation
RGCN weights + root term are dense PE matmuls; small weights replicated.

Self-contained: hardcodes all shapes from the problem spec.
"""
import os
import time

import ml_dtypes
import numpy as np

import concourse.bacc as bacc
import concourse.bass as bass
import concourse.mybir as mybir
import concourse.tile as tile
from concourse.bass_utils import run_bass_kernel_spmd
from concourse.masks import make_identity

# ---------------- problem constants (hardcoded from spec) ----------------
NCORES = 8
N = 50000
E = 800000
R = 5
D = 128
FIN = 768 + 768 + 6 + 11          # 1553 concat input features
FINP = 13 * 128                   # padded to 1664
NLOC = N // NCORES                # 6250
CHUNK = 512                       # nodes per chunk
NCHUNK = 13
NPAD = NCHUNK * CHUNK             # 6656 padded nodes/core
NTAB = NCORES * NPAD              # 53248 table rows
BANK = 512                        # segment columns per PSUM bank
BANKS_PER_CHUNK = CHUNK * R // BANK   # 5
NBANK = NCHUNK * BANKS_PER_CHUNK  # 65
NSEG = NPAD * R                   # 33280 dense segment grid per core
HALFROW = NPAD // 2               # 3328 (only used for tstage staging)
NTABH = (NCORES // 2) * NPAD      # 26624 rows per table view (< 32768)
SLOTS = 128                       # edge slots per tile
COLL_ENG = os.environ.get("KB_COLL_ENG", "scalar")
SINGLE_PACKET = os.environ.get("KB_SP", "1") == "1"
SUBT = int(os.environ.get("KB_SUBT", "4"))    # tiles per gather call
NQUEUE = int(os.environ.get("KB_NQUEUE", "4"))
NLAYER = int(os.environ.get("KB_LAYERS", "4"))
SKIP = set(os.environ.get("KB_SKIP", "").split(","))

F16 = mybir.dt.float16
F32 = mybir.dt.float32
F8 = mybir.dt.float8e4
I16 = mybir.dt.int16
NPF8 = ml_dtypes.float8_e4m3

_CACHE = {}


def _COLL_ENG(nc):
    return {"scalar": nc.scalar, "vector": nc.vector,
            "gpsimd": nc.gpsimd}[COLL_ENG]


# ---------------- host-side graph preprocessing ----------------
def _plan_graph(edge_index, edge_type):
    """Build per-core tile structure. Span layout is shared by all cores
    (SPMD: one program), per-core data (idx, M) differs."""
    src = np.asarray(edge_index[0], dtype=np.int64)
    dst = np.asarray(edge_index[1], dtype=np.int64)
    et = np.asarray(edge_type, dtype=np.int64)

    core = dst // NLOC
    # per-core relabel: sort nodes by in-degree so the 8 cores' per-column
    # count sequences align (tightens max-over-cores span packing)
    deg = np.zeros((NCORES, NLOC), np.int64)
    np.add.at(deg, (core, dst % NLOC), 1)
    order = np.argsort(-deg, axis=1, kind="stable")   # newpos j <- old node
    newpos = np.empty_like(order)
    for k in range(NCORES):
        newpos[k, order[k]] = np.arange(NLOC)

    col = newpos[core, dst % NLOC] * R + et           # 0..31249
    src_core = src // NLOC
    src_loc = newpos[src_core, src % NLOC]
    # rank-major full table [8*NPAD rows]; gather fetches the 512B row PAIR
    # (rows 2j, 2j+1) per request, so the pair index fits int16 and there is
    # a single edge stream; slot parity selects the half via split matmuls
    row = src_core * NPAD + src_loc
    pair = row >> 1
    parity = row & 1

    # per core: edges sorted by col
    edges = {}
    counts = np.zeros((NCORES, NSEG), dtype=np.int64)
    for k in range(NCORES):
        m = core == k
        c = col[m]
        o = np.argsort(c, kind="stable")
        edges[k] = (c[o], pair[m][o], parity[m][o])
        np.add.at(counts[k], c[o], 1)

    invc = 1.0 / np.maximum(counts, 1.0)              # per core

    # static spans per bank: greedy, max-over-cores count <= SLOTS
    spans = []                                        # spans[b] = [widths]
    for b in range(NBANK):
        base = b * BANK
        cc = counts[:, base:base + BANK]              # [NCORES, BANK]
        assert cc.max(initial=0) <= SLOTS, "single segment exceeds tile"
        widths = []
        run = np.zeros(NCORES, dtype=np.int64)
        w = 0
        for j in range(BANK):
            if (run + cc[:, j]).max() > SLOTS:
                widths.append(w)
                run[:] = 0
                w = 0
            run += cc[:, j]
            w += 1
        widths.append(w)
        spans.append(widths)

    ntiles = [len(spans[b]) for b in range(NBANK)]
    # gather-call grouping: calls cover one chunk's tiles
    call_tiles = [sum(ntiles[c * BANKS_PER_CHUNK + b]
                      for b in range(BANKS_PER_CHUNK))
                  for c in range(NCHUNK)]
    tot_tiles = sum(ntiles)

    # per-core data: gather idx (wrapped int16) + 0/1 membership matrices
    # (fp8, exact; separate even/odd-parity planes) + 1/cnt post-scale
    gidx = np.zeros((NCORES, 128, tot_tiles * SLOTS // 16), np.int16)
    mmat = np.zeros((NCORES, 128, 2 * NBANK * BANK), NPF8)
    for k in range(NCORES):
        cols_e, pair_e, par_e = edges[k]
        flat_idx = np.zeros(tot_tiles * SLOTS, np.int16)
        tglob = 0
        for b in range(NBANK):
            base = b * BANK
            lo = 0
            for w in spans[b]:
                e0 = np.searchsorted(cols_e, base + lo)
                e1 = np.searchsorted(cols_e, base + lo + w)
                nslot = e1 - e0
                assert nslot <= SLOTS
                flat_idx[tglob * SLOTS:tglob * SLOTS + nslot] = pair_e[e0:e1]
                mcol = (2 * b + par_e[e0:e1]) * BANK + (cols_e[e0:e1] - base)
                mmat[k, np.arange(nslot), mcol] = NPF8(1.0)
                lo += w
                tglob += 1
        # wrap: element i -> [i%16, i//16], replicated across 8 groups
        wr = flat_idx.reshape(-1, 16).T                # [16, ntot*8]
        gidx[k] = np.tile(wr, (8, 1))
    return dict(spans=spans, ntiles=ntiles, call_tiles=call_tiles,
                tot_tiles=tot_tiles, gidx=gidx, mmat=mmat,
                invc=invc.astype(np.float16), order=order)


# ---------------- device program ----------------
def _build_nc(plan):
    nc = bacc.Bacc("TRN2", target_bir_lowering=False, debug=False,
                   num_devices=NCORES, num_swdge_queues=NQUEUE,
                   dynamic_dma_scratch_size=int(
                       os.environ.get("KB_SCRATCH", "32768")))
    spans, ntiles = plan["spans"], plan["ntiles"]
    call_tiles, tot_tiles = plan["call_tiles"], plan["tot_tiles"]

    # inputs (per core)
    featT = nc.dram_tensor("featT", [FINP, NPAD], F8, kind="ExternalInput")
    w_all = nc.dram_tensor("w_all", [128, 13 * 128], F16, kind="ExternalInput")
    b_x0 = nc.dram_tensor("b_x0", [128, 1], F32, kind="ExternalInput")
    w_in = nc.dram_tensor("w_in", [128, 128], F16, kind="ExternalInput")
    b_in = nc.dram_tensor("b_in", [128, 1], F32, kind="ExternalInput")
    relw = nc.dram_tensor("relw", [128, R * 128], F16, kind="ExternalInput")
    rootw = nc.dram_tensor("rootw", [128, 128], F16, kind="ExternalInput")
    rgcn_b = nc.dram_tensor("rgcn_b", [128, 1], F32, kind="ExternalInput")
    wo1 = nc.dram_tensor("wo1", [128, 128], F16, kind="ExternalInput")
    b_o1 = nc.dram_tensor("b_o1", [128, 1], F32, kind="ExternalInput")
    wo2 = nc.dram_tensor("wo2", [128, 2], F16, kind="ExternalInput")
    b_o2 = nc.dram_tensor("b_o2", [2, 1], F32, kind="ExternalInput")
    gidxA = nc.dram_tensor("gidxA", [128, tot_tiles * 8], I16,
                           kind="ExternalInput")
    mmat = nc.dram_tensor("mmat", [128, 2 * NBANK * BANK], F8,
                          kind="ExternalInput")
    invc_t = nc.dram_tensor("invc_t", [128, NSEG], F16, kind="ExternalInput")
    outT = nc.dram_tensor("outT", [2, NPAD], F32, kind="ExternalOutput")
    DBG = "1" == os.environ.get("KB_DEBUG", "0")
    if DBG:
        dbg_x = nc.dram_tensor("dbg_x", [128, NPAD], F16,
                               kind="ExternalOutput")
        dbg_tab = nc.dram_tensor("dbg_tab", [128, NTAB // 512 * D], F16,
                                 kind="ExternalOutput")
        dbg_gb = nc.dram_tensor("dbg_gb", [128, 8 * D], F16,
                                kind="ExternalOutput")
        dbg_st = nc.dram_tensor("dbg_st", [128, CHUNK * R], F16,
                                kind="ExternalOutput")

    with tile.TileContext(nc) as tc:
        with (
            tc.tile_pool(name="const", bufs=1) as constp,
            tc.tile_pool(name="xt", bufs=2) as xtp,
            tc.tile_pool(name="tst", bufs=1) as tstp,
            tc.tile_pool(name="feat", bufs=3) as featp,
            tc.tile_pool(name="gb", bufs=12) as gbp,
            tc.tile_pool(name="msb", bufs=6) as msbp,
            tc.tile_pool(name="stile", bufs=2) as stp,
            tc.tile_pool(name="small", bufs=3) as smallp,
            tc.tile_pool(name="pbank", bufs=4, space="PSUM") as pbank,
            tc.tile_pool(name="pbig", bufs=2, space="PSUM") as pbig,
            tc.tile_pool(name="ptp", bufs=2, space="PSUM") as ptpp,
            tc.tile_pool(name="dram", bufs=1, space="DRAM") as dramp,
            tc.tile_pool(name="shared", bufs=1, space="DRAM") as sharedp,
        ):
            # ---- resident constants ----
            def load_const(t, shape, dt):
                s = constp.tile(shape, dt, tag=t.name)
                nc.sync.dma_start(s[:], t[:])
                return s
            w_all_s = load_const(w_all, [128, 13 * 128], F16)
            b_x0_s = load_const(b_x0, [128, 1], F32)
            w_in_s = load_const(w_in, [128, 128], F16)
            b_in_s = load_const(b_in, [128, 1], F32)
            relw_s = load_const(relw, [128, R * 128], F16)
            rootw_s = load_const(rootw, [128, 128], F16)
            rgcn_b_s = load_const(rgcn_b, [128, 1], F32)
            wo1_s = load_const(wo1, [128, 128], F16)
            b_o1_s = load_const(b_o1, [128, 1], F32)
            wo2_s = load_const(wo2, [128, 2], F16)
            b_o2_s = load_const(b_o2, [2, 1], F32)
            gidx_s = load_const(gidxA, [128, tot_tiles * 8], I16)
            ident = constp.tile([128, 128], F16, tag="ident")
            make_identity(nc, ident[:])
            invc_sb = constp.tile([128, NSEG], F16, tag="invc")


            # ---- table-build helpers (interleaved into chunk loops so the
            # AllGather of each half fires as soon as its rows exist) ----
            def emit_table_chunk(xsrc, c, tstage):
                for j in range(4 * c, 4 * c + 4):
                    pt = ptpp.tile([128, 128], F16, space="PSUM", tag="ptp")
                    nc.tensor.transpose(pt[:], xsrc[:, j * 128:(j + 1) * 128],
                                        ident[:])
                    nc.vector.tensor_copy(
                        tstage[:, j * 128:(j + 1) * 128], pt[:])

            def new_tsh(layer):
                return dramp.tile([NPAD, D], F16, tag=f"tsh{layer}",
                                  name=f"tsh{layer}")

            def emit_tsh_chunk(tsh, c, tstage):
                # ship chunk c's transposed rows to DRAM incrementally so
                # only the last 128KB DMA sits on the boundary chain
                nc.sync.dma_start(
                    tsh[c * CHUNK:(c + 1) * CHUNK, :].rearrange(
                        "(j p) d -> p j d", p=128),
                    tstage[:, c * CHUNK:(c + 1) * CHUNK].rearrange(
                        "p (j d) -> p j d", d=D))

            def emit_table_ship(layer, tsh):
                # single full-table AllGather (rank-major [8*NPAD, D]); the
                # gather addresses 512B row pairs so idx fits int16
                tb = sharedp.tile([NCORES * NPAD, D], F16,
                                  addr_space="Shared", tag=f"table{layer}")
                if "coll" not in SKIP:
                    nc.gpsimd.collective_compute(
                        "AllGather", mybir.AluOpType.bypass,
                        replica_groups=[list(range(NCORES))],
                        ins=[tsh[:].opt()], outs=[tb[:].opt()])
                else:
                    nc.sync.dma_start(tb[0:NPAD, :], tsh[:])
                # 512B row-pair view: idx j fetches rows (2j, 2j+1)
                return tb[:].rearrange("(a two) d -> a (two d)", two=2)

            def emit_head_chunk(xsrc, c):
                cs = slice(c * CHUNK, (c + 1) * CHUNK)
                p1 = pbig.tile([128, CHUNK], F32, space="PSUM", tag="pbig")
                nc.tensor.matmul(p1[:], lhsT=wo1_s[:], rhs=xsrc[:, cs],
                                 start=True, stop=True)
                hh = smallp.tile([128, CHUNK], F16, tag="x0")
                nc.scalar.activation(hh[:], p1[:],
                                     mybir.ActivationFunctionType.Lrelu,
                                     bias=b_o1_s[:], scale=1.0, alpha=0.01)
                p2 = ptpp.tile([2, CHUNK], F32, space="PSUM", tag="ptp")
                nc.tensor.matmul(p2[:], lhsT=wo2_s[:], rhs=hh[:],
                                 start=True, stop=True)
                ot = smallp.tile([2, CHUNK], F32, tag="ot")
                nc.scalar.activation(ot[:], p2[:],
                                     mybir.ActivationFunctionType.Identity,
                                     bias=b_o2_s[:], scale=1.0)
                nc.sync.dma_start(outT[:, cs], ot[:])

            # tiny warm-up AllGather: absorbs ncfw first-collective staging
            # while the input projection streams in
            if "coll" not in SKIP:
                wag_i = dramp.tile([128, 8], F16, tag="wag_i")
                wag_o = sharedp.tile([NCORES * 128, 8], F16,
                                     addr_space="Shared", tag="wag_o")
                nc.gpsimd.collective_compute(
                    "AllGather", mybir.AluOpType.bypass,
                    replica_groups=[list(range(NCORES))],
                    ins=[wag_i[:].opt()], outs=[wag_o[:].opt()])

            # ---- input projection -> xT [128, NPAD] fp16 (+ layer-0 table)
            xT = xtp.tile([128, NPAD], F16, tag="xT")
            tstage = tstp.tile([128, NPAD], F16, tag="tstage")
            tsh = new_tsh(0)
            tables = []
            for c in range(NCHUNK):
                cs = slice(c * CHUNK, (c + 1) * CHUNK)
                p0 = pbig.tile([128, CHUNK], F32, space="PSUM", tag="pbig")
                ft = featp.tile([128, 13, CHUNK], F8, tag="feat")
                nc.sync.dma_start(
                    ft[:], featT[:, cs].rearrange("(f p) c -> p f c", p=128))
                for f in range(13):
                    nc.tensor.matmul(p0[:],
                                     lhsT=w_all_s[:, f * 128:(f + 1) * 128],
                                     rhs=ft[:, f, :], start=(f == 0),
                                     stop=(f == 12))
                x0 = smallp.tile([128, CHUNK], F16, tag="x0")
                nc.scalar.activation(x0[:], p0[:],
                                     mybir.ActivationFunctionType.Lrelu,
                                     bias=b_x0_s[:], scale=1.0, alpha=0.01)
                p1 = pbig.tile([128, CHUNK], F32, space="PSUM", tag="pbig")
                nc.tensor.matmul(p1[:], lhsT=w_in_s[:], rhs=x0[:],
                                 start=True, stop=True)
                nc.scalar.activation(xT[:, cs], p1[:],
                                     mybir.ActivationFunctionType.Lrelu,
                                     bias=b_in_s[:], scale=1.0, alpha=0.01)
                emit_table_chunk(xT, c, tstage)
                emit_tsh_chunk(tsh, c, tstage)
                if c == NCHUNK - 1:
                    tables = emit_table_ship(0, tsh)
            # big resident 1/cnt table loads after the input-proj DMAs so it
            # overlaps the first AllGather instead of delaying projection
            nc.sync.dma_start(invc_sb[:], invc_t[:])
            if DBG:
                nc.sync.dma_start(dbg_x[:], xT[:])

            # ---- RGCN layers ----
            for layer in range(NLAYER):
                xTn = xtp.tile([128, NPAD], F16, tag="xT")
                build_next = layer < NLAYER - 1
                if build_next:
                    tstage = tstp.tile([128, NPAD], F16, tag="tstage")
                    tsh = new_tsh(layer + 1)
                    next_tables = []
                goffs = 0                 # gather idx cursor
                callno = 0                # round-robin queue cursor
                for c in range(NCHUNK):
                    # gather: sub-calls of <= SUBT tiles each; each slot
                    # fetches a 512B row pair (2 nodes)
                    tc_s = call_tiles[c]
                    gtiles = []
                    for t0 in range(0, tc_s, SUBT):
                        nt = min(SUBT, tc_s - t0)
                        gb = gbp.tile([128, SUBT, 2 * D], F16, tag="gb")
                        ni = nt * SLOTS
                        qn = callno % NQUEUE
                        callno += 1
                        if "gather" not in SKIP:
                            nc.gpsimd.dma_gather(
                                gb[:, :nt, :], tables,
                                gidx_s[:, goffs:goffs + ni // 16],
                                ni, ni, 2 * D, queue_num=qn,
                                single_packet=SINGLE_PACKET)
                        else:
                            nc.vector.memset(gb[:, :nt, :], 0.0)
                        goffs += ni // 16
                        gtiles.append(gb)
                    st = stp.tile([128, CHUNK * R], F16, tag="stile")
                    for b in range(BANKS_PER_CHUNK):
                        bg = c * BANKS_PER_CHUNK + b
                        pb = pbank.tile([128, BANK], F32, space="PSUM",
                                        tag="pbank")
                        n_mm = 2 * len(spans[bg])
                        i_mm = 0
                        msb = msbp.tile([128, 2 * BANK], F8, tag="msb")
                        # issue on the Activation HWDGE ring so mmat
                        # prefetch never queues behind featT/tsh issues
                        # on the SP ring
                        nc.scalar.dma_start(
                            msb[:], mmat[:, 2 * bg * BANK:
                                         2 * (bg + 1) * BANK])
                        lo = 0
                        tloc = sum(ntiles[c * BANKS_PER_CHUNK + bb]
                                   for bb in range(b))
                        for w in spans[bg]:
                            gt = gtiles[tloc // SUBT]
                            for par in (0, 1):
                                nc.tensor.matmul(
                                    pb[:, lo:lo + w],
                                    lhsT=gt[:, tloc % SUBT,
                                            par * D:(par + 1) * D],
                                    rhs=msb[:, par * BANK + lo:
                                            par * BANK + lo + w],
                                    start=(i_mm == 0),
                                    stop=(i_mm == n_mm - 1))
                                i_mm += 1
                            lo += w
                            tloc += 1
                        assert lo == BANK
                        assert i_mm == n_mm
                        nc.vector.scalar_tensor_tensor(
                            st[:, b * BANK:(b + 1) * BANK], pb[:], 1.0,
                            invc_sb[:, bg * BANK:(bg + 1) * BANK],
                            mybir.AluOpType.mult, mybir.AluOpType.mult)
                    if DBG and layer == 0 and c == 0:
                        nc.sync.dma_start(dbg_st[:], st[:])
                    # phase 2: per-relation + root matmuls
                    cs = slice(c * CHUNK, (c + 1) * CHUNK)
                    po = pbig.tile([128, CHUNK], F32, space="PSUM", tag="pbig")
                    str_ap = st[:].rearrange("p (n r) -> p r n", r=R)
                    for r in range(R):
                        nc.tensor.matmul(po[:],
                                         lhsT=relw_s[:, r * 128:(r + 1) * 128],
                                         rhs=str_ap[:, r, :],
                                         start=(r == 0), stop=False)
                    nc.tensor.matmul(po[:], lhsT=rootw_s[:], rhs=xT[:, cs],
                                     start=False, stop=True)
                    nc.scalar.activation(xTn[:, cs], po[:],
                                         mybir.ActivationFunctionType.Identity,
                                         bias=rgcn_b_s[:], scale=1.0)
                    if build_next:
                        emit_table_chunk(xTn, c, tstage)
                        emit_tsh_chunk(tsh, c, tstage)
                        if c == NCHUNK - 1:
                            next_tables = emit_table_ship(layer + 1, tsh)
                    else:
                        emit_head_chunk(xTn, c)
                xT = xTn
                if build_next:
                    tables = next_tables

    nc.compile()
    return nc


# ---------------- host wrapper ----------------
def _pack_inputs(inputs, plan):
    f16 = np.float16
    des, tweet = inputs["des"], inputs["tweet"]
    num_prop, cat_prop = inputs["num_prop"], inputs["cat_prop"]

    w_blk = np.zeros((FINP, 128), np.float32)
    w_blk[0:768, 0:32] = inputs["W_des"]
    w_blk[768:1536, 32:64] = inputs["W_tw"]
    w_blk[1536:1542, 64:96] = inputs["W_np"]
    w_blk[1542:1553, 96:128] = inputs["W_cp"]
    # pack lhsT blocks: [128, 13*128], block f = rows [f*128,(f+1)*128)
    w_all = np.concatenate([w_blk[f * 128:(f + 1) * 128, :]
                            for f in range(13)], axis=1).astype(f16)
    b_x0 = np.concatenate([inputs["b_des"], inputs["b_tw"],
                           inputs["b_np"], inputs["b_cp"]]
                          ).astype(np.float32).reshape(128, 1)
    relw = np.concatenate([inputs["rel_w"][r] for r in range(R)],
                          axis=1).astype(f16)

    in_maps = []
    invc_rep = {}
    for k in range(NCORES):
        invc_rep[k] = np.ascontiguousarray(
            np.broadcast_to(plan["invc"][k][None, :], (128, NSEG)))
    for k in range(NCORES):
        # relabeled node order: device-local column j holds original node
        # k*NLOC + order[k][j]
        gl = k * NLOC + plan["order"][k]
        feat = np.zeros((FINP, NPAD), NPF8)
        feat[0:768, :NLOC] = des[gl].T.astype(NPF8)
        feat[768:1536, :NLOC] = tweet[gl].T.astype(NPF8)
        feat[1536:1542, :NLOC] = num_prop[gl].T.astype(NPF8)
        feat[1542:1553, :NLOC] = cat_prop[gl].T.astype(NPF8)
        m = {
            "featT": feat,
            "w_all": w_all,
            "b_x0": b_x0,
            "w_in": inputs["W_in"].astype(f16),
            "b_in": inputs["b_in"].astype(np.float32).reshape(128, 1),
            "relw": relw,
            "rootw": inputs["root_w"].astype(f16),
            "rgcn_b": inputs["rgcn_b"].astype(np.float32).reshape(128, 1),
            "wo1": inputs["W_o1"].astype(f16),
            "b_o1": inputs["b_o1"].astype(np.float32).reshape(128, 1),
            "wo2": inputs["W_o2"].astype(f16),
            "b_o2": inputs["b_o2"].astype(np.float32).reshape(2, 1),
            "gidxA": plan["gidx"][k],
            "mmat": plan["mmat"][k],
            "invc_t": invc_rep[k],
        }
        in_maps.append(m)
    return in_maps


def _get_compiled(edge_index, edge_type):
    key = hash((np.asarray(edge_index).tobytes(),
                np.asarray(edge_type).tobytes()))
    if key not in _CACHE:
        t0 = time.time()
        plan = _plan_graph(edge_index, edge_type)
        t1 = time.time()
        nc = _build_nc(plan)
        t2 = time.time()
        print(f"[kernel] plan {t1-t0:.0f}s, build+compile {t2-t1:.0f}s",
              flush=True)
        _CACHE[key] = (nc, plan)
    return _CACHE[key]


def kernel(trace=False, **inputs):
    nc, plan = _get_compiled(inputs["edge_index"], inputs["edge_type"])
    in_maps = _pack_inputs(inputs, plan)
    t0 = time.time()
    res = run_bass_kernel_spmd(nc, in_maps, list(range(NCORES)), trace=trace)
    print(f"[kernel] run {time.time()-t0:.0f}s", flush=True)
    out = np.zeros((N, 2), np.float32)
    for k in range(NCORES):
        gl = k * NLOC + plan["order"][k]
        out[gl] = res.results[k]["outT"][:, :NLOC].T
    if trace:
        return out, res
    return out



# revision 74
# speedup vs baseline: 1.0458x; 1.0458x over previous
"""BotRGCN (4 shared RGCN layers) on 8 TRN2 NeuronCores via Bass/Tile.

Strategy (sharding_hint): nodes sharded across 8 cores (6250 each, padded to
6656 = 13*512); edges partitioned by destination core and sorted by
(dst_local, rel) segment; per layer an AllGather replicates the row-major x
table (fp16) to every core's DRAM, then each core dma_gathers its edges'
source rows and computes segment means via PE matmuls against 0/1*(1/cnt)
membership matrices (built host-side; graph is static so the full tiling
is baked into the compiled program, identical across cores). Per-relation
RGCN weights + root term are dense PE matmuls; small weights replicated.

Self-contained: hardcodes all shapes from the problem spec.
"""
import os
import time

import ml_dtypes
import numpy as np

import concourse.bacc as bacc
import concourse.bass as bass
import concourse.mybir as mybir
import concourse.tile as tile
from concourse.bass_utils import run_bass_kernel_spmd
from concourse.masks import make_identity

# ---------------- problem constants (hardcoded from spec) ----------------
NCORES = 8
N = 50000
E = 800000
R = 5
D = 128
FIN = 768 + 768 + 6 + 11          # 1553 concat input features
FINP = 13 * 128                   # padded to 1664
NLOC = N // NCORES                # 6250
CHUNK = 512                       # nodes per chunk
NCHUNK = 13
NPAD = NCHUNK * CHUNK             # 6656 padded nodes/core
NTAB = NCORES * NPAD              # 53248 table rows
BANK = 512                        # segment columns per PSUM bank
BANKS_PER_CHUNK = CHUNK * R // BANK   # 5
NBANK = NCHUNK * BANKS_PER_CHUNK  # 65
NSEG = NPAD * R                   # 33280 dense segment grid per core
HALFROW = NPAD // 2               # 3328 (only used for tstage staging)
NTABH = (NCORES // 2) * NPAD      # 26624 rows per table view (< 32768)
SLOTS = 128                       # edge slots per tile
COLL_ENG = os.environ.get("KB_COLL_ENG", "scalar")
SINGLE_PACKET = os.environ.get("KB_SP", "1") == "1"
SUBT = int(os.environ.get("KB_SUBT", "4"))    # tiles per gather call
NQUEUE = int(os.environ.get("KB_NQUEUE", "4"))
NLAYER = int(os.environ.get("KB_LAYERS", "4"))
SKIP = set(os.environ.get("KB_SKIP", "").split(","))

F16 = mybir.dt.float16
F32 = mybir.dt.float32
F8 = mybir.dt.float8e4
I16 = mybir.dt.int16
NPF8 = ml_dtypes.float8_e4m3

_CACHE = {}


def _COLL_ENG(nc):
    return {"scalar": nc.scalar, "vector": nc.vector,
            "gpsimd": nc.gpsimd}[COLL_ENG]


# ---------------- host-side graph preprocessing ----------------
def _plan_graph(edge_index, edge_type):
    """Build per-core tile structure. Span layout is shared by all cores
    (SPMD: one program), per-core data (idx, M) differs."""
    src = np.asarray(edge_index[0], dtype=np.int64)
    dst = np.asarray(edge_index[1], dtype=np.int64)
    et = np.asarray(edge_type, dtype=np.int64)

    core = dst // NLOC
    # per-core relabel: sort nodes by in-degree so the 8 cores' per-column
    # count sequences align (tightens max-over-cores span packing)
    deg = np.zeros((NCORES, NLOC), np.int64)
    np.add.at(deg, (core, dst % NLOC), 1)
    order = np.argsort(-deg, axis=1, kind="stable")   # newpos j <- old node
    newpos = np.empty_like(order)
    for k in range(NCORES):
        newpos[k, order[k]] = np.arange(NLOC)

    col = newpos[core, dst % NLOC] * R + et           # 0..31249
    src_core = src // NLOC
    src_loc = newpos[src_core, src % NLOC]
    # rank-major full table [8*NPAD rows]; gather fetches the 512B row PAIR
    # (rows 2j, 2j+1) per request, so the pair index fits int16 and there is
    # a single edge stream; slot parity selects the half via split matmuls
    row = src_core * NPAD + src_loc
    pair = row >> 1
    parity = row & 1

    # per core: edges sorted by col
    edges = {}
    counts = np.zeros((NCORES, NSEG), dtype=np.int64)
    for k in range(NCORES):
        m = core == k
        c = col[m]
        o = np.argsort(c, kind="stable")
        edges[k] = (c[o], pair[m][o], parity[m][o])
        np.add.at(counts[k], c[o], 1)

    invc = 1.0 / np.maximum(counts, 1.0)              # per core

    # static spans per bank: greedy, max-over-cores count <= SLOTS
    spans = []                                        # spans[b] = [widths]
    for b in range(NBANK):
        base = b * BANK
        cc = counts[:, base:base + BANK]              # [NCORES, BANK]
        if cc.sum() == 0:
            # destination-padding bank (cols >= NLOC*R on every core):
            # emit nothing; its st columns only feed pad outputs
            spans.append([])
            continue
        assert cc.max(initial=0) <= SLOTS, "single segment exceeds tile"
        widths = []
        run = np.zeros(NCORES, dtype=np.int64)
        w = 0
        for j in range(BANK):
            if (run + cc[:, j]).max() > SLOTS:
                widths.append(w)
                run[:] = 0
                w = 0
            run += cc[:, j]
            w += 1
        widths.append(w)
        spans.append(widths)

    ntiles = [len(spans[b]) for b in range(NBANK)]
    # gather-call grouping: calls cover one chunk's tiles
    call_tiles = [sum(ntiles[c * BANKS_PER_CHUNK + b]
                      for b in range(BANKS_PER_CHUNK))
                  for c in range(NCHUNK)]
    tot_tiles = sum(ntiles)

    # per-core data: gather idx (wrapped int16) + 0/1 membership matrices
    # (fp8, exact; separate even/odd-parity planes) + 1/cnt post-scale
    gidx = np.zeros((NCORES, 128, tot_tiles * SLOTS // 16), np.int16)
    mmat = np.zeros((NCORES, 128, 2 * NBANK * BANK), NPF8)
    for k in range(NCORES):
        cols_e, pair_e, par_e = edges[k]
        flat_idx = np.zeros(tot_tiles * SLOTS, np.int16)
        tglob = 0
        for b in range(NBANK):
            base = b * BANK
            lo = 0
            for w in spans[b]:
                e0 = np.searchsorted(cols_e, base + lo)
                e1 = np.searchsorted(cols_e, base + lo + w)
                nslot = e1 - e0
                assert nslot <= SLOTS
                flat_idx[tglob * SLOTS:tglob * SLOTS + nslot] = pair_e[e0:e1]
                mcol = (2 * b + par_e[e0:e1]) * BANK + (cols_e[e0:e1] - base)
                mmat[k, np.arange(nslot), mcol] = NPF8(1.0)
                lo += w
                tglob += 1
        # wrap: element i -> [i%16, i//16], replicated across 8 groups
        wr = flat_idx.reshape(-1, 16).T                # [16, ntot*8]
        gidx[k] = np.tile(wr, (8, 1))
    return dict(spans=spans, ntiles=ntiles, call_tiles=call_tiles,
                tot_tiles=tot_tiles, gidx=gidx, mmat=mmat,
                invc=invc.astype(np.float16), order=order)


# ---------------- device program ----------------
def _build_nc(plan):
    nc = bacc.Bacc("TRN2", target_bir_lowering=False, debug=False,
                   num_devices=NCORES, num_swdge_queues=NQUEUE,
                   dynamic_dma_scratch_size=int(
                       os.environ.get("KB_SCRATCH", "32768")))
    spans, ntiles = plan["spans"], plan["ntiles"]
    call_tiles, tot_tiles = plan["call_tiles"], plan["tot_tiles"]

    # inputs (per core)
    featT = nc.dram_tensor("featT", [FINP, NPAD], F8, kind="ExternalInput")
    w_all = nc.dram_tensor("w_all", [128, 13 * 128], F16, kind="ExternalInput")
    b_x0 = nc.dram_tensor("b_x0", [128, 1], F32, kind="ExternalInput")
    w_in = nc.dram_tensor("w_in", [128, 128], F16, kind="ExternalInput")
    b_in = nc.dram_tensor("b_in", [128, 1], F32, kind="ExternalInput")
    relw = nc.dram_tensor("relw", [128, R * 128], F16, kind="ExternalInput")
    rootw = nc.dram_tensor("rootw", [128, 128], F16, kind="ExternalInput")
    rgcn_b = nc.dram_tensor("rgcn_b", [128, 1], F32, kind="ExternalInput")
    wo1 = nc.dram_tensor("wo1", [128, 128], F16, kind="ExternalInput")
    b_o1 = nc.dram_tensor("b_o1", [128, 1], F32, kind="ExternalInput")
    wo2 = nc.dram_tensor("wo2", [128, 2], F16, kind="ExternalInput")
    b_o2 = nc.dram_tensor("b_o2", [2, 1], F32, kind="ExternalInput")
    gidxA = nc.dram_tensor("gidxA", [128, tot_tiles * 8], I16,
                           kind="ExternalInput")
    mmat = nc.dram_tensor("mmat", [128, 2 * NBANK * BANK], F8,
                          kind="ExternalInput")
    invc_t = nc.dram_tensor("invc_t", [128, NSEG], F16, kind="ExternalInput")
    outT = nc.dram_tensor("outT", [2, NPAD], F32, kind="ExternalOutput")
    DBG = "1" == os.environ.get("KB_DEBUG", "0")
    if DBG:
        dbg_x = nc.dram_tensor("dbg_x", [128, NPAD], F16,
                               kind="ExternalOutput")
        dbg_tab = nc.dram_tensor("dbg_tab", [128, NTAB // 512 * D], F16,
                                 kind="ExternalOutput")
        dbg_gb = nc.dram_tensor("dbg_gb", [128, 8 * D], F16,
                                kind="ExternalOutput")
        dbg_st = nc.dram_tensor("dbg_st", [128, CHUNK * R], F16,
                                kind="ExternalOutput")

    with tile.TileContext(nc) as tc:
        with (
            tc.tile_pool(name="const", bufs=1) as constp,
            tc.tile_pool(name="xt", bufs=2) as xtp,
            tc.tile_pool(name="tst", bufs=1) as tstp,
            tc.tile_pool(name="feat", bufs=3) as featp,
            tc.tile_pool(name="gb", bufs=12) as gbp,
            tc.tile_pool(name="msb", bufs=6) as msbp,
            tc.tile_pool(name="stile", bufs=2) as stp,
            tc.tile_pool(name="small", bufs=3) as smallp,
            tc.tile_pool(name="pbank", bufs=4, space="PSUM") as pbank,
            tc.tile_pool(name="pbig", bufs=2, space="PSUM") as pbig,
            tc.tile_pool(name="ptp", bufs=2, space="PSUM") as ptpp,
            tc.tile_pool(name="dram", bufs=1, space="DRAM") as dramp,
            tc.tile_pool(name="shared", bufs=1, space="DRAM") as sharedp,
        ):
            # ---- resident constants ----
            def load_const(t, shape, dt):
                s = constp.tile(shape, dt, tag=t.name)
                nc.sync.dma_start(s[:], t[:])
                return s
            w_all_s = load_const(w_all, [128, 13 * 128], F16)
            b_x0_s = load_const(b_x0, [128, 1], F32)
            w_in_s = load_const(w_in, [128, 128], F16)
            b_in_s = load_const(b_in, [128, 1], F32)
            relw_s = load_const(relw, [128, R * 128], F16)
            rootw_s = load_const(rootw, [128, 128], F16)
            rgcn_b_s = load_const(rgcn_b, [128, 1], F32)
            wo1_s = load_const(wo1, [128, 128], F16)
            b_o1_s = load_const(b_o1, [128, 1], F32)
            wo2_s = load_const(wo2, [128, 2], F16)
            b_o2_s = load_const(b_o2, [2, 1], F32)
            gidx_s = load_const(gidxA, [128, tot_tiles * 8], I16)
            ident = constp.tile([128, 128], F16, tag="ident")
            make_identity(nc, ident[:])
            invc_sb = constp.tile([128, NSEG], F16, tag="invc")


            # ---- table-build helpers (interleaved into chunk loops so the
            # AllGather of each half fires as soon as its rows exist) ----
            def emit_table_chunk(xsrc, c, tstage):
                for j in range(4 * c, 4 * c + 4):
                    pt = ptpp.tile([128, 128], F16, space="PSUM", tag="ptp")
                    nc.tensor.transpose(pt[:], xsrc[:, j * 128:(j + 1) * 128],
                                        ident[:])
                    nc.vector.tensor_copy(
                        tstage[:, j * 128:(j + 1) * 128], pt[:])

            def new_tsh(layer):
                return dramp.tile([NPAD, D], F16, tag=f"tsh{layer}",
                                  name=f"tsh{layer}")

            def emit_tsh_chunk(tsh, c, tstage):
                # ship chunk c's transposed rows to DRAM incrementally so
                # only the last 128KB DMA sits on the boundary chain
                nc.sync.dma_start(
                    tsh[c * CHUNK:(c + 1) * CHUNK, :].rearrange(
                        "(j p) d -> p j d", p=128),
                    tstage[:, c * CHUNK:(c + 1) * CHUNK].rearrange(
                        "p (j d) -> p j d", d=D))

            def emit_table_ship(layer, tsh):
                # single full-table AllGather (rank-major [8*NPAD, D]); the
                # gather addresses 512B row pairs so idx fits int16
                tb = sharedp.tile([NCORES * NPAD, D], F16,
                                  addr_space="Shared", tag=f"table{layer}")
                if "coll" not in SKIP:
                    nc.gpsimd.collective_compute(
                        "AllGather", mybir.AluOpType.bypass,
                        replica_groups=[list(range(NCORES))],
                        ins=[tsh[:].opt()], outs=[tb[:].opt()])
                else:
                    nc.sync.dma_start(tb[0:NPAD, :], tsh[:])
                # 512B row-pair view: idx j fetches rows (2j, 2j+1)
                return tb[:].rearrange("(a two) d -> a (two d)", two=2)

            def emit_head_chunk(xsrc, c):
                cs = slice(c * CHUNK, (c + 1) * CHUNK)
                p1 = pbig.tile([128, CHUNK], F32, space="PSUM", tag="pbig")
                nc.tensor.matmul(p1[:], lhsT=wo1_s[:], rhs=xsrc[:, cs],
                                 start=True, stop=True)
                hh = smallp.tile([128, CHUNK], F16, tag="x0")
                nc.scalar.activation(hh[:], p1[:],
                                     mybir.ActivationFunctionType.Lrelu,
                                     bias=b_o1_s[:], scale=1.0, alpha=0.01)
                p2 = ptpp.tile([2, CHUNK], F32, space="PSUM", tag="ptp")
                nc.tensor.matmul(p2[:], lhsT=wo2_s[:], rhs=hh[:],
                                 start=True, stop=True)
                ot = smallp.tile([2, CHUNK], F32, tag="ot")
                nc.scalar.activation(ot[:], p2[:],
                                     mybir.ActivationFunctionType.Identity,
                                     bias=b_o2_s[:], scale=1.0)
                nc.sync.dma_start(outT[:, cs], ot[:])

            # tiny warm-up AllGather: absorbs ncfw first-collective staging
            # while the input projection streams in
            if "coll" not in SKIP:
                wag_i = dramp.tile([128, 8], F16, tag="wag_i")
                wag_o = sharedp.tile([NCORES * 128, 8], F16,
                                     addr_space="Shared", tag="wag_o")
                nc.gpsimd.collective_compute(
                    "AllGather", mybir.AluOpType.bypass,
                    replica_groups=[list(range(NCORES))],
                    ins=[wag_i[:].opt()], outs=[wag_o[:].opt()])

            # ---- input projection -> xT [128, NPAD] fp16 (+ layer-0 table)
            xT = xtp.tile([128, NPAD], F16, tag="xT")
            tstage = tstp.tile([128, NPAD], F16, tag="tstage")
            tsh = new_tsh(0)
            tables = []
            for c in range(NCHUNK):
                cs = slice(c * CHUNK, (c + 1) * CHUNK)
                p0 = pbig.tile([128, CHUNK], F32, space="PSUM", tag="pbig")
                ft = featp.tile([128, 13, CHUNK], F8, tag="feat")
                nc.sync.dma_start(
                    ft[:], featT[:, cs].rearrange("(f p) c -> p f c", p=128))
                for f in range(13):
                    nc.tensor.matmul(p0[:],
                                     lhsT=w_all_s[:, f * 128:(f + 1) * 128],
                                     rhs=ft[:, f, :], start=(f == 0),
                                     stop=(f == 12))
                x0 = smallp.tile([128, CHUNK], F16, tag="x0")
                nc.scalar.activation(x0[:], p0[:],
                                     mybir.ActivationFunctionType.Lrelu,
                                     bias=b_x0_s[:], scale=1.0, alpha=0.01)
                p1 = pbig.tile([128, CHUNK], F32, space="PSUM", tag="pbig")
                nc.tensor.matmul(p1[:], lhsT=w_in_s[:], rhs=x0[:],
                                 start=True, stop=True)
                nc.scalar.activation(xT[:, cs], p1[:],
                                     mybir.ActivationFunctionType.Lrelu,
                                     bias=b_in_s[:], scale=1.0, alpha=0.01)
                emit_table_chunk(xT, c, tstage)
                emit_tsh_chunk(tsh, c, tstage)
                if c == NCHUNK - 1:
                    tables = emit_table_ship(0, tsh)
            # big resident 1/cnt table loads after the input-proj DMAs so it
            # overlaps the first AllGather instead of delaying projection
            nc.sync.dma_start(invc_sb[:], invc_t[:])
            if DBG:
                nc.sync.dma_start(dbg_x[:], xT[:])

            # ---- RGCN layers ----
            for layer in range(NLAYER):
                xTn = xtp.tile([128, NPAD], F16, tag="xT")
                build_next = layer < NLAYER - 1
                if build_next:
                    tstage = tstp.tile([128, NPAD], F16, tag="tstage")
                    tsh = new_tsh(layer + 1)
                    next_tables = []
                goffs = 0                 # gather idx cursor
                callno = 0                # round-robin queue cursor
                for c in range(NCHUNK):
                    # gather: sub-calls of <= SUBT tiles each; each slot
                    # fetches a 512B row pair (2 nodes)
                    tc_s = call_tiles[c]
                    gtiles = []
                    for t0 in range(0, tc_s, SUBT):
                        nt = min(SUBT, tc_s - t0)
                        gb = gbp.tile([128, SUBT, 2 * D], F16, tag="gb")
                        ni = nt * SLOTS
                        qn = callno % NQUEUE
                        callno += 1
                        if "gather" not in SKIP:
                            nc.gpsimd.dma_gather(
                                gb[:, :nt, :], tables,
                                gidx_s[:, goffs:goffs + ni // 16],
                                ni, ni, 2 * D, queue_num=qn,
                                single_packet=SINGLE_PACKET)
                        else:
                            nc.vector.memset(gb[:, :nt, :], 0.0)
                        goffs += ni // 16
                        gtiles.append(gb)
                    st = stp.tile([128, CHUNK * R], F16, tag="stile")
                    for b in range(BANKS_PER_CHUNK):
                        bg = c * BANKS_PER_CHUNK + b
                        if not spans[bg]:
                            continue
                        pb = pbank.tile([128, BANK], F32, space="PSUM",
                                        tag="pbank")
                        n_mm = 2 * len(spans[bg])
                        i_mm = 0
                        msb = msbp.tile([128, 2 * BANK], F8, tag="msb")
                        nc.sync.dma_start(
                            msb[:], mmat[:, 2 * bg * BANK:
                                         2 * (bg + 1) * BANK])
                        lo = 0
                        tloc = sum(ntiles[c * BANKS_PER_CHUNK + bb]
                                   for bb in range(b))
                        for w in spans[bg]:
                            gt = gtiles[tloc // SUBT]
                            for par in (0, 1):
                                nc.tensor.matmul(
                                    pb[:, lo:lo + w],
                                    lhsT=gt[:, tloc % SUBT,
                                            par * D:(par + 1) * D],
                                    rhs=msb[:, par * BANK + lo:
                                            par * BANK + lo + w],
                                    start=(i_mm == 0),
                                    stop=(i_mm == n_mm - 1))
                                i_mm += 1
                            lo += w
                            tloc += 1
                        assert lo == BANK
                        assert i_mm == n_mm
                        nc.vector.scalar_tensor_tensor(
                            st[:, b * BANK:(b + 1) * BANK], pb[:], 1.0,
                            invc_sb[:, bg * BANK:(bg + 1) * BANK],
                            mybir.AluOpType.mult, mybir.AluOpType.mult)
                    if DBG and layer == 0 and c == 0:
                        nc.sync.dma_start(dbg_st[:], st[:])
                    # phase 2: per-relation + root matmuls
                    cs = slice(c * CHUNK, (c + 1) * CHUNK)
                    po = pbig.tile([128, CHUNK], F32, space="PSUM", tag="pbig")
                    str_ap = st[:].rearrange("p (n r) -> p r n", r=R)
                    for r in range(R):
                        nc.tensor.matmul(po[:],
                                         lhsT=relw_s[:, r * 128:(r + 1) * 128],
                                         rhs=str_ap[:, r, :],
                                         start=(r == 0), stop=False)
                    nc.tensor.matmul(po[:], lhsT=rootw_s[:], rhs=xT[:, cs],
                                     start=False, stop=True)
                    nc.scalar.activation(xTn[:, cs], po[:],
                                         mybir.ActivationFunctionType.Identity,
                                         bias=rgcn_b_s[:], scale=1.0)
                    if build_next:
                        emit_table_chunk(xTn, c, tstage)
                        emit_tsh_chunk(tsh, c, tstage)
                        if c == NCHUNK - 1:
                            next_tables = emit_table_ship(layer + 1, tsh)
                    else:
                        emit_head_chunk(xTn, c)
                xT = xTn
                if build_next:
                    tables = next_tables

    nc.compile()
    return nc


# ---------------- host wrapper ----------------
def _pack_inputs(inputs, plan):
    f16 = np.float16
    des, tweet = inputs["des"], inputs["tweet"]
    num_prop, cat_prop = inputs["num_prop"], inputs["cat_prop"]

    w_blk = np.zeros((FINP, 128), np.float32)
    w_blk[0:768, 0:32] = inputs["W_des"]
    w_blk[768:1536, 32:64] = inputs["W_tw"]
    w_blk[1536:1542, 64:96] = inputs["W_np"]
    w_blk[1542:1553, 96:128] = inputs["W_cp"]
    # pack lhsT blocks: [128, 13*128], block f = rows [f*128,(f+1)*128)
    w_all = np.concatenate([w_blk[f * 128:(f + 1) * 128, :]
                            for f in range(13)], axis=1).astype(f16)
    b_x0 = np.concatenate([inputs["b_des"], inputs["b_tw"],
                           inputs["b_np"], inputs["b_cp"]]
                          ).astype(np.float32).reshape(128, 1)
    relw = np.concatenate([inputs["rel_w"][r] for r in range(R)],
                          axis=1).astype(f16)

    in_maps = []
    invc_rep = {}
    for k in range(NCORES):
        invc_rep[k] = np.ascontiguousarray(
            np.broadcast_to(plan["invc"][k][None, :], (128, NSEG)))
    for k in range(NCORES):
        # relabeled node order: device-local column j holds original node
        # k*NLOC + order[k][j]
        gl = k * NLOC + plan["order"][k]
        feat = np.zeros((FINP, NPAD), NPF8)
        feat[0:768, :NLOC] = des[gl].T.astype(NPF8)
        feat[768:1536, :NLOC] = tweet[gl].T.astype(NPF8)
        feat[1536:1542, :NLOC] = num_prop[gl].T.astype(NPF8)
        feat[1542:1553, :NLOC] = cat_prop[gl].T.astype(NPF8)
        m = {
            "featT": feat,
            "w_all": w_all,
            "b_x0": b_x0,
            "w_in": inputs["W_in"].astype(f16),
            "b_in": inputs["b_in"].astype(np.float32).reshape(128, 1),
            "relw": relw,
            "rootw": inputs["root_w"].astype(f16),
            "rgcn_b": inputs["rgcn_b"].astype(np.float32).reshape(128, 1),
            "wo1": inputs["W_o1"].astype(f16),
            "b_o1": inputs["b_o1"].astype(np.float32).reshape(128, 1),
            "wo2": inputs["W_o2"].astype(f16),
            "b_o2": inputs["b_o2"].astype(np.float32).reshape(2, 1),
            "gidxA": plan["gidx"][k],
            "mmat": plan["mmat"][k],
            "invc_t": invc_rep[k],
        }
        in_maps.append(m)
    return in_maps


def _get_compiled(edge_index, edge_type):
    key = hash((np.asarray(edge_index).tobytes(),
                np.asarray(edge_type).tobytes()))
    if key not in _CACHE:
        t0 = time.time()
        plan = _plan_graph(edge_index, edge_type)
        t1 = time.time()
        nc = _build_nc(plan)
        t2 = time.time()
        print(f"[kernel] plan {t1-t0:.0f}s, build+compile {t2-t1:.0f}s",
              flush=True)
        _CACHE[key] = (nc, plan)
    return _CACHE[key]


def kernel(trace=False, **inputs):
    nc, plan = _get_compiled(inputs["edge_index"], inputs["edge_type"])
    in_maps = _pack_inputs(inputs, plan)
    t0 = time.time()
    res = run_bass_kernel_spmd(nc, in_maps, list(range(NCORES)), trace=trace)
    print(f"[kernel] run {time.time()-t0:.0f}s", flush=True)
    out = np.zeros((N, 2), np.float32)
    for k in range(NCORES):
        gl = k * NLOC + plan["order"][k]
        out[gl] = res.results[k]["outT"][:, :NLOC].T
    if trace:
        return out, res
    return out

